# revision 36
# baseline (speedup 1.0000x reference)
"""Trainium2 Bass kernel for nn_MultiHeadAttention_39986145526235.

Reference computation (per batch b):
    q = Q @ W_Q.T, k = K @ W_K.T, v = V @ W_V.T   (split into H=16 heads of dh=64)
    s1 = q @ U_h.T            [S, R=12]  per head
    s2 = s1.T @ k             [R, dh]
    attn = softmax(s2, -1)
    ctx = attn @ v.T          [R, S]
    out = LayerNorm(ctx_flat @ W_lin.T + b_lin)

Key algebraic restructure (R=12 << D=1024 makes the projections collapsible):
    s1    = Q @ (U_h @ W_Qh).T          -- q never materialized
    s2    = (s1.T @ K) @ W_Kh.T         -- k never materialized
    ctx   = (attn @ W_Vh) @ V.T         -- v never materialized
This reduces total work from ~55 GFLOP to ~10 GFLOP.

Precision: the s2 logits have std ~530 with min top1-top2 gap ~0.5, so the
logit chain (s1, T=s1.T@K, s2) runs in fp32 matmuls; the post-softmax chain
(ctx, final linear) runs in float32r (fast fp32, ~12-bit mantissa), which
gives ~2e-4 absmax relative error end-to-end.

Sharding: 8 cores = batch (4) x head-half (2). No collectives; each core
computes 192 of the 1536 output rows.

Performance notes (production cost model, per core): ~115 us span; DMA is
the binding resource (~33 MB per core at ~360 GB/s = 97 us), PE ~80 us.
DMA count is minimized (each dma_start costs ~625 ns of HWDGE descriptor
generation): inputs are host-packed so the whole kernel issues ~40 DMAs.
Post-stream loads are ordered by consumption (wkt -> wv -> xtv -> wlt) so
the s2/softmax/MT/ctx/final chain overlaps the remaining transfers.
"""

import ml_dtypes
import numpy as np

import concourse.bass as bass
import concourse.mybir as mybir
import concourse.tile as tile
from concourse import bacc
from concourse.bass_utils import run_bass_kernel_spmd
from concourse.masks import make_identity

B, S, D, H, DH, R = 4, 2048, 1024, 16, 64, 12
HL = 8  # heads per core
ROWS = 192  # output rows per core
EPS = 1e-6
F32 = mybir.dt.float32
F32R = mybir.dt.float32r
BF16 = mybir.dt.bfloat16
CTX_BF16 = False  # bf16 for the post-softmax chain (V, W_V, W_lin): ~3e-3 vs ~3e-4
AF = mybir.ActivationFunctionType
NST = S // 128  # 16 s-tiles
NCT = D // 128  # 8 contraction tiles


def _build_nc(debug_taps=False, trivial_affine=False):
    nc = bacc.Bacc(None, target_bir_lowering=False)
    dbg = {}
    if debug_taps:
        for nm, shape in [
            ("d_s1_all", [16, 128, 96]),
            ("d_tt0", [128, 384]),
            ("d_tt1", [128, 384]),
            ("d_s2", [96, 512]),
            ("d_attn", [96, 512]),
            ("d_nmx", [96, 1]),
            ("d_den", [96, 1]),
            ("d_ex", [96, 512]),
            ("d_bdT0", [128, 96]),
            ("d_mt0", [128, 384]),
            ("d_mt1", [128, 384]),
            ("d_ctx", [96, 2048]),
            ("d_ctxT0", [128, 192]),
            ("d_osb0", [128, 1024]),
        ]:
            dbg[nm] = nc.declare_dram_parameter(nm, shape, F32, isOutput=True)

    # packed layouts, one DMA per tensor (or per 2-s-tile group for xq/xk):
    #   xq[g][p, stl*1024 + ct*128 + s] = Q[b][(2g+stl)*128 + s, ct*128 + p]
    #   xk[g][p, stl*1024 + d]          = K[b][(2g+stl)*128 + p, d]
    #   xtv[p, ct*2048 + s]             = V[b][s, ct*128 + p]
    #   at[p, ct*96 + col]              = A_all[col, ct*128 + p]
    #   wkt[p, ct*512 + q]              = WKT_local[ct*128 + p, q]
    #   wlt[p, cc*1024 + j]             = W_lin[j, cc*128 + p]
    xtq = nc.declare_dram_parameter("xtq", [8, 128, 2 * D], F32, isOutput=False)
    xk = nc.declare_dram_parameter("xk", [8, 128, 2 * D], F32, isOutput=False)
    CDT = BF16 if CTX_BF16 else F32R
    WVDT = BF16 if CTX_BF16 else F32
    xtv = nc.declare_dram_parameter("xtv", [128, NCT * S], CDT, isOutput=False)
    at = nc.declare_dram_parameter("at", [128, NCT * 96], F32, isOutput=False)
    wkt = nc.declare_dram_parameter("wkt", [128, NCT * 512], F32, isOutput=False)
    wv = nc.declare_dram_parameter("wv", [128, 4 * D], WVDT, isOutput=False)
    smask = nc.declare_dram_parameter("smask", [96, 512], F32, isOutput=False)
    wlt = nc.declare_dram_parameter("wlt", [128, NCT * D], CDT, isOutput=False)
    if not trivial_affine:
        raws = nc.declare_dram_parameter("raws", [1, 3 * D], F32, isOutput=False)
    out = nc.declare_dram_parameter("out", [ROWS, D], F32, isOutput=True)

    xtq_ap, xk_ap = xtq.ap(), xk.ap()

    with tile.TileContext(nc) as tc:
        with tc.tile_pool(name="glob", bufs=1) as glob:
            ident = glob.tile([128, 128], F32, name="ident")
            make_identity(nc, ident[:])
            # prefetch ACT function tables (Exp/Square/Sqrt) while ACT is
            # idle, so no LoadActFuncSet lands on the kernel tail.
            actwarm = glob.tile([1, 4], F32, name="actwarm")
            nc.vector.memset(actwarm[:], 1.0)
            nc.scalar.activation(actwarm[:, 0:1], actwarm[:, 0:1], AF.Exp)
            nc.scalar.activation(actwarm[:, 1:2], actwarm[:, 1:2], AF.Square)
            nc.scalar.activation(actwarm[:, 2:3], actwarm[:, 2:3], AF.Sqrt)

            wlt_sb = glob.tile([128, NCT * D], CDT, name="wlt")
            xtv_sb = glob.tile([128, NCT * S], CDT, name="xtv")
            mtsb = [
                glob.tile([128, 384], CDT, name=f"mtsb{i}") for i in range(2)
            ]

            with (
                tc.tile_pool(name="poolA", bufs=1) as poolA,
                tc.tile_pool(name="xq_pool", bufs=3) as xq_pool,
                tc.tile_pool(name="xk_pool", bufs=3) as xk_pool,
                tc.tile_pool(name="s1_pool", bufs=3) as s1_pool,
            ):
                at_sb = poolA.tile([128, NCT * 96], F32, name="at")
                nc.sync.dma_start(at_sb[:], at.ap())
                smask_sb = poolA.tile([96, 512], F32, name="smask")
                wkt_sb = poolA.tile([128, NCT * 512], F32, name="wkt")
                wv_sb = poolA.tile([128, 4 * D], WVDT, name="wv")

                # ---- phase 1: s1 then TT (T transposed), streaming over s
                with (
                    tc.tile_pool(name="pp_tt", bufs=1, space="PSUM") as pp_tt,
                    tc.tile_pool(name="pp_s1", bufs=2, space="PSUM") as pp_s1,
                ):
                    tt_ps = [
                        pp_tt.tile([128, 384], F32, name=f"tt_ps{i}")
                        for i in range(2)
                    ]
                    # 4 accumulation slices share each bank; matmul start=True
                    # clears has_written for the WHOLE bank, so zero the banks
                    # once and accumulate with start=False throughout.
                    nc.vector.memset(tt_ps[0][:], 0.0)
                    nc.vector.memset(tt_ps[1][:], 0.0)
                    for g in range(8):
                        xq_t = xq_pool.tile([128, 2 * D], F32, name="xq_t")
                        nc.sync.dma_start(xq_t[:, :D], xtq_ap[g, :, :D])
                        xk_t = xk_pool.tile([128, 2 * D], F32, name="xk_t")
                        nc.sync.dma_start(xk_t[:, :D], xk_ap[g, :, :D])
                        nc.sync.dma_start(xq_t[:, D:], xtq_ap[g, :, D:])
                        nc.sync.dma_start(xk_t[:, D:], xk_ap[g, :, D:])

                        for stl in range(2):
                            st = 2 * g + stl
                            s1_ps = pp_s1.tile([128, 96], F32, name="s1_ps")
                            for ct in range(NCT):
                                nc.tensor.matmul(
                                    s1_ps[:],
                                    xq_t[:, stl * D + ct * 128 : stl * D + (ct + 1) * 128],
                                    at_sb[:, ct * 96 : (ct + 1) * 96],
                                    start=(ct == 0),
                                    stop=(ct == NCT - 1),
                                )
                            s1_sb = s1_pool.tile([128, 96], F32, name="s1_sb")
                            nc.vector.tensor_copy(s1_sb[:], s1_ps[:])
                            if debug_taps:
                                nc.sync.dma_start(dbg["d_s1_all"].ap()[st], s1_sb[:])
                            for ct in range(NCT):
                                bank, off = divmod(ct, 4)
                                nc.tensor.matmul(
                                    tt_ps[bank][:, off * 96 : (off + 1) * 96],
                                    xk_t[:, stl * D + ct * 128 : stl * D + (ct + 1) * 128],
                                    s1_sb[:],
                                    start=False,
                                    stop=(st == NST - 1),
                                    skip_group_check=True,
                                )
                    # post-stream loads: s2/MT inputs first, then wlt (final's
                    # input, cheap to land early), then xtv last (ctx chains
                    # directly after its halves arrive).
                    nc.sync.dma_start(smask_sb[:], smask.ap())
                    nc.sync.dma_start(wkt_sb[:, : 4 * 512], wkt.ap()[:, : 4 * 512])
                    nc.sync.dma_start(wkt_sb[:, 4 * 512 :], wkt.ap()[:, 4 * 512 :])
                    nc.sync.dma_start(wv_sb[:], wv.ap())
                    for q in range(4):
                        nc.sync.dma_start(
                            xtv_sb[:, 2 * q * S : 2 * (q + 1) * S],
                            xtv.ap()[:, 2 * q * S : 2 * (q + 1) * S],
                        )
                    nc.sync.dma_start(wlt_sb[:, : 4 * D], wlt.ap()[:, : 4 * D])
                    nc.sync.dma_start(wlt_sb[:, 4 * D :], wlt.ap()[:, 4 * D :])
                    tt_sb = [
                        poolA.tile([128, 384], F32, name=f"tt_sb{i}")
                        for i in range(2)
                    ]
                    nc.vector.tensor_copy(tt_sb[0][:], tt_ps[0][:])
                    nc.vector.tensor_copy(tt_sb[1][:], tt_ps[1][:])
                    if debug_taps:
                        nc.sync.dma_start(dbg["d_tt0"].ap(), tt_sb[0][:])
                        nc.sync.dma_start(dbg["d_tt1"].ap(), tt_sb[1][:])

                # ---- s2, softmax, MT
                with tc.tile_pool(name="pp_a", bufs=1, space="PSUM") as pp_a:
                    s2_ps = pp_a.tile([96, 512], F32, name="s2_ps")
                    for ct in range(NCT):
                        bank, off = divmod(ct, 4)
                        nc.tensor.matmul(
                            s2_ps[:],
                            tt_sb[bank][:, off * 96 : (off + 1) * 96],
                            wkt_sb[:, ct * 512 : (ct + 1) * 512],
                            start=(ct == 0),
                            stop=(ct == NCT - 1),
                        )
                    # masked softmax over the full [96, 512]: the host mask
                    # is 0 on each row's own head block, -1e9 elsewhere, so
                    # exp() zeroes cross-head entries and the row sum is the
                    # correct per-head denominator.
                    s2_sb = poolA.tile([96, 512], F32, name="s2_sb")
                    nc.vector.tensor_add(s2_sb[:], s2_ps[:], smask_sb[:])

                    if debug_taps:
                        nc.sync.dma_start(dbg["d_s2"].ap(), s2_sb[:])
                    nmx = poolA.tile([96, 1], F32, name="nmx")
                    nc.vector.reduce_max(
                        nmx[:], s2_sb[:], axis=mybir.AxisListType.X, negate=True
                    )
                    ex = poolA.tile([96, 512], F32, name="ex")
                    den = poolA.tile([96, 1], F32, name="den")
                    nc.scalar.activation(
                        ex[:], s2_sb[:], AF.Exp, bias=nmx[:], accum_out=den[:]
                    )
                    # normalization deferred: ctx rows are scaled by 1/den at
                    # psum eviction (ctx is linear in attn, rows align).
                    rden = glob.tile([96, 1], F32, name="rden")
                    nc.vector.reciprocal(rden[:], den[:])
                    attn = ex

                    if debug_taps:
                        nc.sync.dma_start(dbg["d_attn"].ap(), attn[:])
                        nc.sync.dma_start(dbg["d_nmx"].ap(), nmx[:])
                        nc.sync.dma_start(dbg["d_den"].ap(), den[:])
                        nc.sync.dma_start(dbg["d_ex"].ap(), ex[:])
                    # attnT chunks: transpose [96, 128] -> [128, 96]; chunk p4
                    # holds heads (2*p4, 2*p4+1) stacked on partitions; the
                    # pair's own 24 attn rows are cols p4*24..p4*24+24.
                    bdT = []
                    for p4 in range(4):
                        attnT_ps = pp_a.tile([128, 96], F32, name="attnT_ps")
                        nc.tensor.transpose(
                            attnT_ps[:],
                            attn[:, p4 * 128 : (p4 + 1) * 128],
                            ident[:96, :96],
                        )
                        t = poolA.tile([128, 96], WVDT, name=f"bdT{p4}")
                        nc.vector.tensor_copy(t[:], attnT_ps[:])
                        bdT.append(t)

                    # MT[c, (h,r)] = sum_dh W_V[h*64+dh, c] * attnT[dh, (h,r)]
                    # pair-packed K=128; zeros in bdT kill cross-head terms.
                    mt_ps = [
                        pp_a.tile([128, 384], F32, name=f"mt_ps{i}")
                        for i in range(2)
                    ]
                    for p4 in range(4):
                        for ct in range(NCT):
                            bank, off = divmod(ct, 4)
                            nc.tensor.matmul(
                                mt_ps[bank][
                                    :, off * 96 + p4 * 24 : off * 96 + (p4 + 1) * 24
                                ],
                                wv_sb[:, p4 * D + ct * 128 : p4 * D + (ct + 1) * 128],
                                bdT[p4][:, p4 * 24 : (p4 + 1) * 24],
                                start=True,
                                stop=True,
                            )
                    nc.vector.tensor_copy(mtsb[0][:], mt_ps[0][:])
                    nc.vector.tensor_copy(mtsb[1][:], mt_ps[1][:])
                    if debug_taps and not CTX_BF16:
                        nc.sync.dma_start(dbg["d_bdT0"].ap(), bdT[0][:])
                        if not CTX_BF16:
                            nc.sync.dma_start(dbg["d_mt0"].ap(), mtsb[0][:].bitcast(F32))
                            nc.sync.dma_start(dbg["d_mt1"].ap(), mtsb[1][:].bitcast(F32))

            # ---- ctx, ctxT, final linear, layernorm
            with (
                tc.tile_pool(name="tailp", bufs=1) as tailp,
                tc.tile_pool(name="pp_ctx", bufs=1, space="PSUM") as pp_ctx,
                tc.tile_pool(name="pp_tr", bufs=2, space="PSUM") as pp_tr,
                tc.tile_pool(name="pp_fin", bufs=2, space="PSUM") as pp_fin,
            ):
                ctx_sb = tailp.tile([96, S], F32, name="ctx_sb")
                # ct-major so the first xtv half (ct 0-3) can start before the
                # second half's DMA lands; 4 pinned psum banks accumulate.
                ctx_ps = [
                    pp_ctx.tile([96, 512], F32, name=f"ctx_ps{c}") for c in range(4)
                ]
                for ct in range(NCT):
                    bank, off = divmod(ct, 4)
                    for chunk in range(4):
                        nc.tensor.matmul(
                            ctx_ps[chunk][:],
                            mtsb[bank][:, off * 96 : (off + 1) * 96],
                            xtv_sb[:, ct * S + chunk * 512 : ct * S + (chunk + 1) * 512],
                            start=(ct == 0),
                            stop=(ct == NCT - 1),
                        )
                for chunk in range(4):
                    nc.vector.tensor_scalar_mul(
                        ctx_sb[:, chunk * 512 : (chunk + 1) * 512],
                        ctx_ps[chunk][:],
                        rden[:],
                    )

                if debug_taps:
                    nc.sync.dma_start(dbg["d_ctx"].ap(), ctx_sb[:])
                ctxT_sb = [
                    tailp.tile([128, 192], CDT, name=f"ctxT{ct}")
                    for ct in range(NCT)
                ]
                for half in range(2):
                    for ct in range(NCT):
                        tr_ps = pp_tr.tile([128, 96], F32, name="tr_ps")
                        nc.tensor.transpose(
                            tr_ps[:],
                            ctx_sb[:, half * 1024 + ct * 128 : half * 1024 + (ct + 1) * 128],
                            ident[:96, :96],
                        )
                        nc.vector.tensor_copy(
                            ctxT_sb[ct][:, half * 96 : (half + 1) * 96], tr_ps[:]
                        )

                if debug_taps and not CTX_BF16:
                    nc.sync.dma_start(dbg["d_ctxT0"].ap(), ctxT_sb[0][:].bitcast(F32))
                if not trivial_affine:
                    # broadcast bias/gamma/beta across partitions
                    raw_sb = tailp.tile([1, 3 * D], F32, name="raw_sb")
                    nc.sync.dma_start(raw_sb[:], raws.ap())
                    b_b = tailp.tile([128, D], F32, name="b_b")
                    g_b = tailp.tile([128, D], F32, name="g_b")
                    be_b = tailp.tile([128, D], F32, name="be_b")
                    nc.gpsimd.partition_broadcast(b_b[:], raw_sb[:, 0:D])
                    nc.gpsimd.partition_broadcast(g_b[:], raw_sb[:, D : 2 * D])
                    nc.gpsimd.partition_broadcast(be_b[:], raw_sb[:, 2 * D : 3 * D])

                # jc-major: wlt arrives as two jc-halves, so the jc=0 matmuls
                # and evictions overlap the jc=1 half's DMA.
                osb_t = [
                    tailp.tile([128, D], F32, name=f"osb{rt}") for rt in range(2)
                ]
                smj_t = [
                    tailp.tile([128, 2], F32, name=f"smj{rt}") for rt in range(2)
                ]
                smq_t = [
                    tailp.tile([128, 2], F32, name=f"smq{rt}") for rt in range(2)
                ]
                sqd_t = [
                    tailp.tile([128, D], F32, name=f"sqd{rt}") for rt in range(2)
                ]
                for jc in range(2):
                    fin_ps = [
                        pp_fin.tile([128, 512], F32, name="fin_ps") for _ in range(2)
                    ]
                    for cc in range(NCT):
                        for rt, rs in ((0, 128), (1, 64)):
                            nc.tensor.matmul(
                                fin_ps[rt][:rs, :],
                                ctxT_sb[cc][:, rt * 128 : rt * 128 + rs],
                                wlt_sb[:, jc * 4 * D + cc * 512 : jc * 4 * D + (cc + 1) * 512],
                                start=(cc == 0),
                                stop=(cc == NCT - 1),
                            )
                    for rt, rs in ((0, 128), (1, 64)):
                        if trivial_affine:
                            nc.scalar.activation(
                                osb_t[rt][:rs, jc * 512 : (jc + 1) * 512],
                                fin_ps[rt][:rs, :],
                                AF.Copy,
                                accum_out=smj_t[rt][:rs, jc : jc + 1],
                            )
                            nc.scalar.activation(
                                sqd_t[rt][:rs, jc * 512 : (jc + 1) * 512],
                                fin_ps[rt][:rs, :],
                                AF.Square,
                                accum_out=smq_t[rt][:rs, jc : jc + 1],
                            )
                        else:
                            nc.vector.tensor_add(
                                osb_t[rt][:rs, jc * 512 : (jc + 1) * 512],
                                fin_ps[rt][:rs, :],
                                b_b[:rs, jc * 512 : (jc + 1) * 512],
                            )
                for rt, rs in ((0, 128), (1, 64)):
                    osb = osb_t[rt][:rs, :]
                    smj = smj_t[rt][:rs, :]
                    if debug_taps and rt == 0:
                        nc.sync.dma_start(dbg["d_osb0"].ap(), osb[:].bitcast(F32))
                    negmean = tailp.tile([128, 1], F32, name=f"negmean{rt}")[:rs, :]
                    vareps = tailp.tile([128, 1], F32, name=f"vareps{rt}")[:rs, :]
                    if trivial_affine:
                        # one-pass stats: sums and sum-of-squares were
                        # accumulated during eviction; var = E[x^2] - mu^2.
                        smq = smq_t[rt][:rs, :]
                        nc.vector.tensor_scalar(
                            negmean,
                            smj[:, 0:1],
                            smj[:, 1:2],
                            -1.0 / D,
                            op0=mybir.AluOpType.add,
                            op1=mybir.AluOpType.mult,
                        )
                        nm2 = tailp.tile([128, 1], F32, name=f"nm2{rt}")[:rs, :]
                        nc.vector.tensor_scalar(
                            nm2,
                            negmean,
                            negmean,
                            -1.0,
                            op0=mybir.AluOpType.mult,
                            op1=mybir.AluOpType.mult,
                        )
                        sq2 = tailp.tile([128, 1], F32, name=f"sq2{rt}")[:rs, :]
                        nc.vector.tensor_scalar(
                            sq2,
                            smq[:, 0:1],
                            smq[:, 1:2],
                            1.0 / D,
                            op0=mybir.AluOpType.add,
                            op1=mybir.AluOpType.mult,
                        )
                        nc.vector.tensor_scalar(
                            vareps,
                            sq2,
                            nm2,
                            float(EPS),
                            op0=mybir.AluOpType.add,
                            op1=mybir.AluOpType.add,
                        )
                    else:
                        sm = tailp.tile([128, 1], F32, name=f"sm{rt}")[:rs, :]
                        nc.vector.reduce_sum(sm, osb[:], axis=mybir.AxisListType.X)
                        nc.scalar.mul(negmean, sm, -1.0 / D)
                        cent_t = tailp.tile([128, D], F32, name=f"cent{rt}")
                        cent = cent_t[:rs, :]
                        nc.vector.tensor_scalar_add(cent, osb[:], negmean)
                        ssum = tailp.tile([128, 1], F32, name=f"ssum{rt}")[:rs, :]
                        nc.scalar.activation(
                            sqd_t[rt][:rs, :], cent, AF.Square, accum_out=ssum
                        )
                        nc.vector.tensor_scalar(
                            vareps,
                            ssum,
                            1.0 / D,
                            float(EPS),
                            op0=mybir.AluOpType.mult,
                            op1=mybir.AluOpType.add,
                        )
                    srt = tailp.tile([128, 1], F32, name=f"srt{rt}")[:rs, :]
                    nc.scalar.activation(srt, vareps, AF.Sqrt)
                    rstd = tailp.tile([128, 1], F32, name=f"rstd{rt}")[:rs, :]
                    nc.vector.reciprocal(rstd, srt)
                    normed = tailp.tile([128, D], F32, name=f"normed{rt}")[:rs, :]
                    if trivial_affine:
                        nc.vector.tensor_scalar(
                            normed,
                            osb[:],
                            negmean,
                            rstd,
                            op0=mybir.AluOpType.add,
                            op1=mybir.AluOpType.mult,
                        )
                    else:
                        nc.vector.tensor_scalar_mul(normed, cent, rstd)
                        nc.vector.tensor_mul(normed, normed, g_b[:rs, :])
                        nc.vector.tensor_add(normed, normed, be_b[:rs, :])
                    nc.sync.dma_start(
                        out.ap()[rt * 128 : rt * 128 + rs, :], normed
                    )

    nc.finalize()
    return nc


_SMASK = np.full((96, 512), -1e9, dtype=np.float32)
for _hp in range(HL):
    _SMASK[_hp * 12 : (_hp + 1) * 12, _hp * 64 : (_hp + 1) * 64] = 0.0

_CACHE = {}


def get_nc(debug_taps=False, trivial_affine=False):
    key = ("nc", debug_taps, trivial_affine)
    if key not in _CACHE:
        _CACHE[key] = _build_nc(debug_taps, trivial_affine)
    return _CACHE[key]


def prep_in_maps(inputs):
    """Build the 8 per-core input maps from full inputs."""
    Q = np.ascontiguousarray(np.asarray(inputs["Q"], dtype=np.float32))
    K = np.ascontiguousarray(np.asarray(inputs["K"], dtype=np.float32))
    V = np.ascontiguousarray(np.asarray(inputs["V"], dtype=np.float32))
    U = np.asarray(inputs["U"], dtype=np.float32)
    WQ = np.asarray(inputs["W_Q"], dtype=np.float32)
    WK = np.asarray(inputs["W_K"], dtype=np.float32)
    WV = np.asarray(inputs["W_V"], dtype=np.float32)
    WL = np.asarray(inputs["W_lin"], dtype=np.float32)
    blin = np.asarray(inputs["b_lin"], dtype=np.float32).reshape(1, D)
    gamma = np.asarray(inputs["gamma"], dtype=np.float32).reshape(1, D)
    beta = np.asarray(inputs["beta"], dtype=np.float32).reshape(1, D)

    WQh = WQ.reshape(H, DH, D)
    WKh = WK.reshape(H, DH, D)
    WVh = WV.reshape(H, DH, D)
    # wlt[p, jc*4096 + cc*512 + j] = W_lin[jc*512 + j, cc*128 + p]
    wlt_full = np.ascontiguousarray(
        WL.T.reshape(NCT, 128, 2, 512).transpose(1, 2, 0, 3)
    ).reshape(128, NCT * D)
    if CTX_BF16:
        wlt_full = wlt_full.astype(ml_dtypes.bfloat16)

    trivial_affine = (
        not blin.any() and not beta.any() and bool((gamma == 1.0).all())
    )
    in_maps = []
    for core in range(8):
        b, hh = divmod(core, 2)
        hs = slice(hh * HL, (hh + 1) * HL)
        # A[h', r, c] = sum_dh U[b, h, r, dh] * W_Q[h*64+dh, c]
        A = np.einsum(
            "hrd,hdc->hrc", U[b, hs].astype(np.float64), WQh[hs].astype(np.float64)
        ).astype(np.float32)
        at_m = np.ascontiguousarray(
            A.reshape(96, D).T.reshape(NCT, 128, 96).transpose(1, 0, 2)
        ).reshape(128, NCT * 96)
        wkt_m = np.ascontiguousarray(
            WKh[hs].reshape(512, D).T.reshape(NCT, 128, 512).transpose(1, 0, 2)
        ).reshape(128, NCT * 512)
        # wv[p, p4*D + c] = W_V[(8*hh + 2*p4 + p//64)*64 + p%64, c]
        wv_m = np.ascontiguousarray(
            WVh[hs].reshape(4, 2 * 64, D).transpose(1, 0, 2).reshape(128, 4 * D)
        )
        if CTX_BF16:
            wv_m = wv_m.astype(ml_dtypes.bfloat16)
        xq_m = np.ascontiguousarray(
            Q[b].reshape(8, 2, 128, NCT, 128).transpose(0, 4, 1, 3, 2)
        ).reshape(8, 128, 2 * D)
        xk_m = np.ascontiguousarray(
            K[b].reshape(8, 2, 128, D).transpose(0, 2, 1, 3)
        ).reshape(8, 128, 2 * D)
        xtv_m = np.ascontiguousarray(
            V[b].T.reshape(NCT, 128, S).transpose(1, 0, 2)
        ).reshape(128, NCT * S)
        if CTX_BF16:
            xtv_m = xtv_m.astype(ml_dtypes.bfloat16)
        in_maps.append(
            {
                "xtq": xq_m,
                "xk": xk_m,
                "xtv": xtv_m,
                "at": at_m,
                "wkt": wkt_m,
                "wv": wv_m,
                "smask": _SMASK,
                "wlt": wlt_full,
            }
        )
        if not trivial_affine:
            in_maps[-1]["raws"] = np.concatenate([blin, gamma, beta], axis=1)
    return in_maps, trivial_affine


def _row_perm(hh):
    """global row index (within batch) for each local row of a core."""
    g = np.empty(ROWS, dtype=np.int64)
    for row_local in range(ROWS):
        chunk, rem = divmod(row_local, 96)
        hp, r = divmod(rem, 12)
        g[row_local] = r * 32 + (HL * hh + hp) * 2 + chunk
    return g


_PERMS = [_row_perm(0), _row_perm(1)]


def gather_output(core_outs):
    out = np.empty((B, 384, D), dtype=np.float32)
    for core in range(8):
        b, hh = divmod(core, 2)
        out[b, _PERMS[hh]] = core_outs[core]
    return out


def kernel(**inputs):
    in_maps, trivial_affine = prep_in_maps(inputs)
    nc = get_nc(trivial_affine=trivial_affine)
    res = run_bass_kernel_spmd(nc, in_maps, list(range(8)))
    return gather_output([res.results[c]["out"] for c in range(8)])
